# revision 9
# baseline (speedup 1.0000x reference)
"""TRN2 Bass kernel for the diffusion-MRI PGSE Monte Carlo simulation.

Reference semantics (jax, default 'rbg' PRNG impl = XLA Philox4x32-10 bits):
  per step i in 0..T-2:
    key_i  = rbg fold_in(key(seed), i)        (threefry2x32 of the half-key)
    bits   = RngBitGenerator(philox) for shape (N,3)
    u      = uniform(lo=nextafter(-1,0), hi=1) from bits
    n_i    = sqrt(2) * erfinv(u)
  V[j]     = sum_i w_i * n_i[j]                (w from the PGSE envelope)
  phase    = B @ V-per-particle;  out_m = |mean_n exp(i*phase_mn)|

Device strategy (8 NeuronCores, SPMD):
  - shard philox blocks (4 uint32 words each) across cores
  - partition dim = step (4 step-blocks of 128), free dim = philox block
  - philox via 16x16 partial products (Pool int32 mults/adds wrap exactly;
    DVE does shifts/bitwise; mulhi assembled carry-free)
  - erfinv: XLA's two-branch polynomial via custom fused DVE ops; ln/sqrt on ACT
  - V = weighted step-sum via PE matmul into PSUM
  - phases = B~ @ V via PE; sin/cos via ADD_RANGE_WRAP x2 + ACT Sin with
    accumulated sums; host combines per-core partial sums.
"""
import math
import numpy as np

GAMMA = 267.513e6
PHILOX_M0 = 0xD2511F53
PHILOX_M1 = 0xCD9E8D57
PHILOX_KB0 = 0x9E3779B9
PHILOX_KB1 = 0xBB67AE85
N_CORES = 8
P = 128

ERFINV_CA = [2.81022636e-08, 3.43273939e-07, -3.5233877e-06,
             -4.39150654e-06, 0.00021858087, -0.00125372503,
             -0.00417768164, 0.246640727, 1.50140941]
ERFINV_CB = [-0.000200214257, 0.000100950558, 0.00134934322,
             -0.00367342844, 0.00573950773, -0.0076224613,
             0.00943887047, 1.00167406, 2.83297682]


# ----------------------------------------------------------------- host math
def _threefry2x32(k1, k2, x0, x1):
    R0 = (13, 15, 26, 6)
    R1 = (17, 29, 16, 24)
    with np.errstate(over='ignore'):
        ks0 = np.uint32(k1); ks1 = np.uint32(k2)
        ks2 = np.uint32(ks0 ^ ks1 ^ np.uint32(0x1BD11BDA))
        x0 = (np.asarray(x0, np.uint32) + ks0).astype(np.uint32)
        x1 = (np.asarray(x1, np.uint32) + ks1).astype(np.uint32)
        ks = (ks0, ks1, ks2)
        for r in range(5):
            for rot in (R0 if r % 2 == 0 else R1):
                x0 = (x0 + x1).astype(np.uint32)
                x1 = ((x1 << np.uint32(rot)) | (x1 >> np.uint32(32 - rot))).astype(np.uint32)
                x1 = x1 ^ x0
            x0 = (x0 + ks[(r + 1) % 3]).astype(np.uint32)
            x1 = (x1 + ks[(r + 2) % 3] + np.uint32(r + 1)).astype(np.uint32)
    return x0, x1


def _step_keys(seed, n_steps):
    k1 = np.uint32((int(seed) >> 32) & 0xFFFFFFFF)
    k2 = np.uint32(int(seed) & 0xFFFFFFFF)
    i = np.arange(n_steps, dtype=np.int64)
    hi = (i >> 32).astype(np.uint32)
    lo = (i & 0xFFFFFFFF).astype(np.uint32)
    f0, f1 = _threefry2x32(k1, k2, hi, lo)
    return f0, f1


def _mulhilo_host(M, c):
    p = np.uint64(M) * np.asarray(c, np.uint64)
    return (p & np.uint64(0xFFFFFFFF)).astype(np.uint32), (p >> np.uint64(32)).astype(np.uint32)


def _compute_weights(delta_us, Delta_us, dt_us):
    dt = float(dt_us) * 1e-6
    delta = float(delta_us) * 1e-6
    Delta = float(Delta_us) * 1e-6
    T_max = Delta + delta + dt
    times = np.arange(0.0, T_max, dt, dtype=np.float32)
    T = len(times)
    rise = np.float32(dt)

    def pulse(t0):
        return (np.clip((times - np.float32(t0)) / rise, 0.0, 1.0)
                - np.clip((times - np.float32(t0) - np.float32(delta)) / rise, 0.0, 1.0)).astype(np.float32)

    env = (pulse(0.0) - pulse(Delta)).astype(np.float32)
    w = np.cumsum(env[::-1].astype(np.float64))[::-1]
    return w[1:].astype(np.float32), T    # w[i] = sum_{t>=i+1} env[t]


# ------------------------------------------------------- custom DVE ops (erfinv)
_OPS_REGISTERED = {}


def _register_erfinv_ops():
    if _OPS_REGISTERED:
        return _OPS_REGISTERED
    from concourse.dve_spec import Spec, Src0, Src1, C0, C1, C2, lower, _spill_c3_to_src1, C3
    from concourse.dve_ops import DveOp, OPS, has_src1, get_dve_sub_opcode
    import concourse.dve_ops as _do
    from concourse.dve_uop import DveOpSpec

    def reg(name, spec):
        op = DveOp(name, spec, subdim=False, uops_sha={})
        OPS.append(op)
        _do._SUB_OPCODE_FOR_NAME[op.name] = _do._CUSTOM_DVE_ROW_BASE + len(OPS) - 1
        _do.CUSTOM_DVE_SPECS[op.name] = op.spec
        for ver in ("v3", "v4"):
            sp = DveOpSpec(name=op.name, opcode=get_dve_sub_opcode(op.name),
                           uops=lower(op.spec, ver=ver), rd1_en=has_src1(op.spec))
            op.uops_sha[ver] = sp.sha(ver)
        return op

    # Branch A head: x = C0 - lnv ; p = (C1*x + C2)*x + C3[spilled]
    xa = C0 - Src0
    _OPS_REGISTERED['HA1'] = reg("ANT_EI_A1", Spec(
        body=_spill_c3_to_src1((C1 * xa + C2) * xa + C3),
        reference=lambda in0, in1, s0, s1, imm2: 0))
    # Continue (A): x = C0 - lnv ; p = (Src1*x + C1)*x + C2
    _OPS_REGISTERED['HC_A'] = reg("ANT_EI_CA", Spec(
        body=(Src1 * xa + C1) * xa + C2,
        reference=lambda in0, in1, s0, s1, imm2: 0))
    # Branch B head: x = sq + C0 ; p = (C1*x + C2)*x + C3[spilled]
    xb = Src0 + C0
    _OPS_REGISTERED['HB1'] = reg("ANT_EI_B1", Spec(
        body=_spill_c3_to_src1((C1 * xb + C2) * xb + C3),
        reference=lambda in0, in1, s0, s1, imm2: 0))
    # Continue (B): x = sq + C0 ; p = (Src1*x + C1)*x + C2
    _OPS_REGISTERED['HC_B'] = reg("ANT_EI_CB", Spec(
        body=(Src1 * xb + C1) * xb + C2,
        reference=lambda in0, in1, s0, s1, imm2: 0))
    return _OPS_REGISTERED


# ---------------------------------------------------------------- bass build
def _build_bass(NB, n_steps, n_part_core, Fc_list, step_blocks, n_meas, dbg=False):
    """Build the per-core SPMD bass program. Returns (nc, names...)."""
    import concourse.bacc as bacc
    import concourse.mybir as mybir
    from concourse.tile import TileContext
    from contextlib import ExitStack

    ops = _register_erfinv_ops()
    A = mybir.AluOpType
    AF = mybir.ActivationFunctionType
    f32 = mybir.dt.float32
    u32 = mybir.dt.uint32

    nblk = len(step_blocks)
    E = 4 * NB                       # elements (words) per core
    assert E == 3 * n_part_core

    nc = bacc.Bacc("TRN2", target_bir_lowering=False, debug=False)

    # const APs for activation biases
    def register_const(value, dtype=f32):
        t = nc.alloc_sbuf_tensor(f"const-{dtype.name}-{value}", [P, 1], dtype)
        nc.gpsimd.memset(t.ap(), value)
        nc.const_aps.aps[(dtype, value)] = t.ap()
    register_const(float(np.pi / 2))
    nc.all_engine_barrier()

    # ---- inputs (per-core data; SPMD shard along axis 0)
    NKC = 21                                        # key columns per step-block
    t_keys = nc.dram_tensor("keys", [P, nblk * NKC], u32, kind="ExternalInput")
    t_wt = nc.dram_tensor("wt", [P, nblk], f32, kind="ExternalInput")
    t_bt = nc.dram_tensor("bt", [65, n_meas], f32, kind="ExternalInput")
    t_mask = nc.dram_tensor("mask64", [65, 64], f32, kind="ExternalInput")
    t_out = nc.dram_tensor("out", [n_meas, 2], f32, kind="ExternalOutput")
    t_dbg = {}
    if dbg:
        Fc0 = Fc_list[0]
        for nm, shape, dt in [("c0i", [P, Fc0], u32), ("hi0r1", [P, Fc0], u32),
                              ("w0", [P, Fc0], u32), ("w1", [P, Fc0], u32),
                              ("w2", [P, Fc0], u32), ("w3", [P, Fc0], u32),
                              ("nrm", [P, 4 * Fc0], f32), ("vs", [65, 6252], f32),
                              ("ph0", [n_meas, 512], f32)]:
            t_dbg[nm] = nc.dram_tensor("dbg_" + nm, shape, dt, kind="ExternalOutput")

    LO_u = np.float32(np.nextafter(np.float32(-1), np.float32(0)))
    D_u = np.float32(1.0) - LO_u
    M0h, M0l = PHILOX_M0 >> 16, PHILOX_M0 & 0xFFFF
    M1h, M1l = PHILOX_M1 >> 16, PHILOX_M1 & 0xFFFF

    ca, cb = [np.float32(x) for x in ERFINV_CA], [np.float32(x) for x in ERFINV_CB]

    with TileContext(nc) as tc, ExitStack() as ctx:
        pool = ctx.enter_context(tc.tile_pool(name="sb", bufs=1))
        psum = ctx.enter_context(tc.tile_pool(name="ps", bufs=1, space="PSUM"))

        def til(shape, dt, tag):
            return pool.tile(shape, dt, name=tag, tag=tag)

        # static inputs in SBUF
        keys = til([P, nblk * NKC], u32, "keys"); nc.sync.dma_start(keys[:], t_keys[:])
        wt = til([P, nblk], f32, "wt"); nc.sync.dma_start(wt[:], t_wt[:])
        bt = til([65, n_meas], f32, "bt"); nc.sync.dma_start(bt[:], t_bt[:])
        mask64 = til([65, 64], f32, "mask64"); nc.sync.dma_start(mask64[:], t_mask[:])

        # small integer consts as [P,1] tiles
        def iconst(tag, val):
            t = til([P, 1], u32, tag)
            nc.gpsimd.memset(t[:], val)
            return t
        s16 = iconst("s16", 16)
        s9 = iconst("s9", 9)
        sOR = iconst("sOR", 0x3F800000)
        mk16 = iconst("mk16", 0xFFFF)
        mhb = {0: iconst("m0h", M0h), 1: iconst("m1h", M1h)}
        ca2t = til([P, 1], f32, "ca2t"); nc.gpsimd.memset(ca2t[:], float(ca[2]))
        cb2t = til([P, 1], f32, "cb2t"); nc.gpsimd.memset(cb2t[:], float(cb[2]))
        mlb = {0: iconst("m0l", M0l), 1: iconst("m1l", M1l)}
        mfull = {0: iconst("m0f", PHILOX_M0), 1: iconst("m1f", PHILOX_M1)}

        # V rows at partitions 0/32/64 (compute-engine partition-base rule);
        # rows in between are zeroed and contracted away by zero B~ rows.
        Vs = til([65, n_part_core], f32, "Vs")
        nc.gpsimd.memset(Vs[:], 0)

        # key column helper: column index within step-block kb
        def KC(kb, idx):
            return keys[:, kb * NKC + idx: kb * NKC + idx + 1]
        # columns: 0=F0B 1=XC2_1 2=XC0_2 3=XC2_2 4=XC2_3 5..12=K0_r(r=2..9) 13..20=K1_r(r=2..9)

        nb_done = 0
        for ci, Fc in enumerate(Fc_list):
            W = 4 * Fc
            pV = psum.tile([1, W], f32, name="pV", tag="pV")
            iot = til([P, Fc], u32, f"iota{ci % 2}")
            nc.gpsimd.iota(iot[:], pattern=[[1, Fc]], base=nb_done, channel_multiplier=0)

            for kb in range(nblk):
                # tags shared across (ci, kb) parity for pipelining
                sl = (ci * nblk + kb) % 2
                t = lambda tag, dt=u32: til([P, Fc], dt, f"{tag}_{sl}")

                def mulhilo(mi, cin, lo_t, hi_t, const_xor=None):
                    """device mulhilo(M_mi, cin). Writes lo->lo_t, hi->hi_t."""
                    ch = t("ch"); cl = t("cl")
                    nc.vector.tensor_single_scalar(ch[:], cin, s16[:], A.logical_shift_right)
                    nc.vector.tensor_single_scalar(cl[:], cin, mk16[:], A.bitwise_and)
                    p11 = t("p11"); p01 = t("p01"); p10 = t("p10"); p00 = t("p00")
                    nc.gpsimd.tensor_tensor(p11[:], ch[:], mhb[mi][:].broadcast_to([P, Fc]), A.mult)
                    nc.gpsimd.tensor_tensor(p01[:], ch[:], mlb[mi][:].broadcast_to([P, Fc]), A.mult)
                    nc.gpsimd.tensor_tensor(p10[:], cl[:], mhb[mi][:].broadcast_to([P, Fc]), A.mult)
                    nc.gpsimd.tensor_tensor(p00[:], cl[:], mlb[mi][:].broadcast_to([P, Fc]), A.mult)
                    nc.gpsimd.tensor_tensor(lo_t[:], cin, mfull[mi][:].broadcast_to([P, Fc]), A.mult)
                    t0 = t("t0")
                    nc.vector.tensor_single_scalar(t0[:], p00[:], s16[:], A.logical_shift_right)
                    s1 = t("s1")
                    nc.gpsimd.tensor_tensor(s1[:], p01[:], t0[:], A.add)
                    if mi == 0:
                        s2 = t("s2")
                        nc.gpsimd.tensor_tensor(s2[:], s1[:], p10[:], A.add)
                        h1 = t("h1")
                        nc.vector.tensor_single_scalar(h1[:], s2[:], s16[:], A.logical_shift_right)
                        nc.gpsimd.tensor_tensor(hi_t[:], p11[:], h1[:], A.add)
                    else:
                        pl = t("pl"); ph = t("ph")
                        nc.vector.tensor_single_scalar(pl[:], p10[:], mk16[:], A.bitwise_and)
                        nc.vector.tensor_single_scalar(ph[:], p10[:], s16[:], A.logical_shift_right)
                        s2 = t("s2")
                        nc.gpsimd.tensor_tensor(s2[:], s1[:], pl[:], A.add)
                        h1 = t("h1")
                        nc.vector.tensor_single_scalar(h1[:], s2[:], s16[:], A.logical_shift_right)
                        ha = t("s1t")
                        nc.gpsimd.tensor_tensor(ha[:], p11[:], h1[:], A.add)
                        nc.gpsimd.tensor_tensor(hi_t[:], ha[:], ph[:], A.add)

                # state tiles: ping-pong 2 tags per line
                c0 = t("c0A"); c2 = t("c2A")
                lo0 = t("lo0A"); lo1 = t("lo1A")
                hi0 = t("hi0"); hi1 = t("hi1")
                alt = {"c0A": "c0B", "c0B": "c0A", "c2A": "c2B", "c2B": "c2A",
                       "lo0A": "lo0B", "lo0B": "lo0A", "lo1A": "lo1B", "lo1B": "lo1A"}
                tagof = {id(c0): "c0A", id(c2): "c2A", id(lo0): "lo0A", id(lo1): "lo1A"}

                def nxt(cur):
                    nt = t(alt[tagof[id(cur)]])
                    tagof[id(nt)] = alt[tagof[id(cur)]]
                    return nt

                # c0_0 = iota + F0B
                nc.gpsimd.tensor_tensor(c0[:], iot[:], KC(kb, 0).broadcast_to([P, Fc]), A.add)

                if dbg and ci == 0 and kb == 0:
                    nc.sync.dma_start(t_dbg["c0i"][:], c0[:])
                # round 1: device M0-mulhilo; c2_1 = hi0 ^ XC2_1 ; c3_1 = lo0
                mulhilo(0, c0[:], lo0, hi0)
                if dbg and ci == 0 and kb == 0:
                    nc.sync.dma_start(t_dbg["hi0r1"][:], hi0[:])
                nc.vector.tensor_single_scalar(c2[:], hi0[:], KC(kb, 1), A.bitwise_xor)
                # round 2: device M1-mulhilo on c2_1; c0_2 = hi1 ^ XC0_2; c1_2 = lo1;
                #          c2_2 = c3_1 ^ XC2_2 ; c3_2 = const (folded into XC2_3)
                mulhilo(1, c2[:], lo1, hi1)
                nc0 = nxt(c0)
                nc.vector.tensor_single_scalar(nc0[:], hi1[:], KC(kb, 2), A.bitwise_xor)
                nc2 = nxt(c2)
                nc.vector.tensor_single_scalar(nc2[:], lo0[:], KC(kb, 3), A.bitwise_xor)
                c0, c2 = nc0, nc2
                # lo1 currently = c1_2

                for r in range(2, 10):
                    nlo0 = nxt(lo0); nlo1 = nxt(lo1)
                    mulhilo(0, c0[:], nlo0, hi0)
                    mulhilo(1, c2[:], nlo1, hi1)
                    nc0 = nxt(c0)
                    # c0' = hi1 ^ c1 ^ k0_r ; c1 = lo1 (prev)
                    nc.vector.scalar_tensor_tensor(nc0[:], lo1[:], KC(kb, 5 + (r - 2)), hi1[:],
                                                   A.bitwise_xor, A.bitwise_xor)
                    nc2 = nxt(c2)
                    if r == 2:
                        # c3_2 is a host const folded into XC2_3 = lo0c2 ^ k1_2
                        nc.vector.tensor_single_scalar(nc2[:], hi0[:], KC(kb, 4), A.bitwise_xor)
                    else:
                        nc.vector.scalar_tensor_tensor(nc2[:], lo0[:], KC(kb, 13 + (r - 2)), hi0[:],
                                                       A.bitwise_xor, A.bitwise_xor)
                    c0, c2, lo0, lo1 = nc0, nc2, nlo0, nlo1

                # final words: (c0_10, c1_10=lo1, c2_10, c3_10=lo0)
                words = [c0, lo1, c2, lo0]
                if dbg and ci == 0 and kb == 0:
                    for wi, wt_ in enumerate(words):
                        nc.sync.dma_start(t_dbg[f"w{wi}"][:], wt_[:])
                normals = til([P, W], f32, f"norm_{sl}")
                for w, wt_tile in enumerate(words):
                    cv = t("p11")
                    nc.vector.tensor_scalar(cv[:], wt_tile[:], s9[:], sOR[:],
                                            A.logical_shift_right, A.bitwise_or)
                    u0 = t("ph", f32)
                    nc.vector.tensor_single_scalar(u0[:], cv[:].bitcast(f32), -1.0, A.add)
                    u = t("u", f32)
                    nc.vector.tensor_scalar(u[:], u0[:], float(D_u), float(LO_u),
                                            A.mult, A.add)
                    u2 = t("p01", f32)
                    nc.scalar.activation(u2[:], u[:], AF.Square)
                    lnr = t("s1t", f32)
                    nc.scalar.activation(lnr[:], u2[:], AF.Ln, bias=1.0, scale=-1.0)
                    lnv = t("lnv", f32)
                    nc.vector.tensor_single_scalar(lnv[:], lnr[:], 0.0, A.min)
                    sq = t("p10", f32)
                    nc.scalar.activation(sq[:], lnv[:], AF.Sqrt, scale=-1.0)
                    pa = t("s1", f32); pb2 = t("t0", f32)
                    nc.vector._custom_dve(ops['HA1'], out=pa[:], in0=lnv[:],
                                          s0=-2.5, s1=float(ca[0]), imm2=float(ca[1]),
                                          in1=ca2t[:])
                    pa2 = t("s2", f32)
                    nc.vector._custom_dve(ops['HC_A'], out=pa2[:], in0=lnv[:], in1=pa[:],
                                          s0=-2.5, s1=float(ca[3]), imm2=float(ca[4]))
                    nc.vector._custom_dve(ops['HC_A'], out=pa[:], in0=lnv[:], in1=pa2[:],
                                          s0=-2.5, s1=float(ca[5]), imm2=float(ca[6]))
                    nc.vector._custom_dve(ops['HC_A'], out=pa2[:], in0=lnv[:], in1=pa[:],
                                          s0=-2.5, s1=float(ca[7]), imm2=float(ca[8]))
                    nc.vector._custom_dve(ops['HB1'], out=pb2[:], in0=sq[:],
                                          s0=-3.0, s1=float(cb[0]), imm2=float(cb[1]),
                                          in1=cb2t[:])
                    pb3 = t("h1", f32)
                    nc.vector._custom_dve(ops['HC_B'], out=pb3[:], in0=sq[:], in1=pb2[:],
                                          s0=-3.0, s1=float(cb[3]), imm2=float(cb[4]))
                    nc.vector._custom_dve(ops['HC_B'], out=pb2[:], in0=sq[:], in1=pb3[:],
                                          s0=-3.0, s1=float(cb[5]), imm2=float(cb[6]))
                    nc.vector._custom_dve(ops['HC_B'], out=pb3[:], in0=sq[:], in1=pb2[:],
                                          s0=-3.0, s1=float(cb[7]), imm2=float(cb[8]))
                    msk = t("pl", f32)
                    nc.vector.tensor_single_scalar(msk[:], lnv[:], -5.0, A.is_gt)
                    dd = t("ph", f32)
                    nc.vector.tensor_sub(dd[:], pa2[:], pb3[:])
                    nc.vector.tensor_mul(dd[:], dd[:], msk[:])
                    nc.vector.tensor_add(dd[:], dd[:], pb3[:])
                    nc.vector.tensor_tensor(normals[:, w::4], dd[:], u[:], A.mult)

                if dbg and ci == 0 and kb == 0:
                    nc.sync.dma_start(t_dbg["nrm"][:], normals[:])
                # matmul: V += wt_kb . normals  (accumulate over step-blocks)
                off = 0
                while off < W:
                    fw = min(512, W - off)
                    nc.tensor.matmul(out=pV[0:1, off:off + fw], lhsT=wt[:, kb:kb + 1],
                                     rhs=normals[:, off:off + fw],
                                     start=(kb == 0), stop=(kb == nblk - 1))
                    off += fw

            # deinterleave psum V chunk into Vs[d, :]
            base_j = 4 * nb_done
            for d in range(3):
                q0 = (d - base_j) % 3
                cnt = len(range(q0, W, 3))
                n0 = (base_j + q0) // 3
                nc.vector.tensor_copy(Vs[32 * d:32 * d + 1, n0:n0 + cnt], pV[0:1, q0::3])
            nb_done += Fc

        # mask the pad particles (last <=64 columns)
        nc.vector.tensor_mul(Vs[:, n_part_core - 64:], Vs[:, n_part_core - 64:], mask64[:])

        if dbg:
            nc.sync.dma_start(t_dbg["vs"][:], Vs[:, :6252])
        # phases + trig sums
        nchunks = [min(512, n_part_core - o) for o in range(0, n_part_core, 512)]
        ncol = len(nchunks)
        sincol = til([n_meas, ncol], f32, "sincol")
        coscol = til([n_meas, ncol], f32, "coscol")
        scr = til([n_meas, 512], f32, "scr")
        off = 0
        for qi, cw in enumerate(nchunks):
            pP = psum.tile([n_meas, cw], f32, name=f"pP{qi % 2}", tag=f"pP{qi % 2}")
            nc.tensor.matmul(out=pP[:, :], lhsT=bt[:, :], rhs=Vs[:, off:off + cw],
                             start=True, stop=True)
            if dbg and qi == 0:
                dscr = til([n_meas, 512], f32, "dscr")
                nc.vector.tensor_copy(dscr[:, :cw], pP[:, :])
                nc.sync.dma_start(t_dbg["ph0"][:], dscr[:])
            r1 = til([n_meas, cw], f32, f"r1_{qi % 2}")
            nc.vector.add_range_wrap(r1[:], pP[:, :], 0.0, float(np.pi), float(2 * np.pi))
            r2 = til([n_meas, cw], f32, f"r2_{qi % 2}")
            nc.vector.add_range_wrap(r2[:], r1[:], 0.0, float(np.pi), float(2 * np.pi))
            nc.scalar.activation(scr[:, :cw], r2[:], mybir.ActivationFunctionType.Sin,
                                 accum_out=sincol[:, qi:qi + 1])
            q1 = til([n_meas, cw], f32, f"q1_{qi % 2}")
            nc.vector.add_range_wrap(q1[:], pP[:, :], float(np.pi / 2), float(np.pi), float(2 * np.pi))
            q2 = til([n_meas, cw], f32, f"q2_{qi % 2}")
            nc.vector.add_range_wrap(q2[:], q1[:], 0.0, float(np.pi), float(2 * np.pi))
            nc.scalar.activation(scr[:, :cw], q2[:], mybir.ActivationFunctionType.Sin,
                                 accum_out=coscol[:, qi:qi + 1])
            off += cw

        res2 = til([n_meas, 2], f32, "res2")
        nc.vector.tensor_reduce(res2[:, 0:1], coscol[:], axis=mybir.AxisListType.X,
                                op=mybir.AluOpType.add)
        nc.vector.tensor_reduce(res2[:, 1:2], sincol[:], axis=mybir.AxisListType.X,
                                op=mybir.AluOpType.add)
        nc.sync.dma_start(t_out[:], res2[:])

    nc.finalize()
    return nc


# --------------------------------------------------------------- run wrapper
_CACHE = {}


def _get_runner(cfg):
    if cfg in _CACHE:
        return _CACHE[cfg]
    NB, n_steps, n_part_core, Fc_list, step_blocks, n_meas, dbg = cfg
    nc = _build_bass(NB, n_steps, n_part_core, list(Fc_list), list(step_blocks), n_meas, dbg=dbg)

    import time
    import jax
    from jax.sharding import Mesh, PartitionSpec
    from jax.experimental.shard_map import shard_map
    import concourse.mybir as mybir
    from concourse.bass2jax import _bass_exec_p, install_neuronx_cc_hook, partition_id_tensor

    install_neuronx_cc_hook()
    partition_name = nc.partition_id_tensor.name if nc.partition_id_tensor else None
    in_names, out_names, out_avals, zero_outs = [], [], [], []
    for alloc in nc.m.functions[0].allocations:
        if not isinstance(alloc, mybir.MemoryLocationSet):
            continue
        name = alloc.memorylocations[0].name
        if alloc.kind == "ExternalInput":
            if name != partition_name:
                in_names.append(name)
        elif alloc.kind == "ExternalOutput":
            shape = tuple(alloc.tensor_shape)
            dtype = mybir.dt.np(alloc.dtype)
            out_names.append(name)
            out_avals.append(jax.core.ShapedArray(shape, dtype))
            zero_outs.append(np.zeros(shape, dtype))
    n_params = len(in_names)
    n_outs = len(out_avals)
    all_in = list(in_names) + list(out_names)
    if partition_name is not None:
        all_in.append(partition_name)

    def _body(*args):
        operands = list(args)
        if partition_name is not None:
            operands.append(partition_id_tensor())
        outs = _bass_exec_p.bind(
            *operands, out_avals=tuple(out_avals), in_names=tuple(all_in),
            out_names=tuple(out_names), lowering_input_output_aliases=(),
            sim_require_finite=True, sim_require_nnan=True, nc=nc)
        return tuple(outs)

    devices = jax.devices()[:N_CORES]
    mesh = Mesh(np.asarray(devices), ("core",))
    fn = jax.jit(shard_map(_body, mesh=mesh,
                           in_specs=(PartitionSpec("core"),) * (n_params + n_outs),
                           out_specs=(PartitionSpec("core"),) * n_outs,
                           check_rep=False), keep_unused=True)
    runner = dict(fn=fn, in_names=in_names, out_names=out_names,
                  out_avals=out_avals, zero_outs=zero_outs)
    _CACHE[cfg] = runner
    return runner


def kernel(G_amps, gradients, D_long, D_trans, delta_us, Delta_us, dt_us,
           N_particles, seed):
    G_amps = np.asarray(G_amps, np.float32)
    gradients = np.asarray(gradients, np.float32)
    n_meas = G_amps.shape[0]
    N = int(N_particles)

    w, T = _compute_weights(delta_us, Delta_us, dt_us)
    n_steps = T - 1
    dt = np.float32(float(dt_us) * 1e-6)
    sig = np.sqrt(2.0 * dt * np.stack([np.float32(D_trans), np.float32(D_trans),
                                       np.float32(D_long)])).astype(np.float32)
    B = (np.float32(GAMMA) * dt * G_amps[:, None] * gradients) * sig[None, :] \
        * np.float32(np.sqrt(2.0))
    B = B.astype(np.float32)

    # ---- philox keys and derived per-step constants
    f0, f1 = _step_keys(seed, n_steps)
    NBTOT = (3 * N + 11) // 12          # philox blocks covering 3N words
    NBTOT = -(-(3 * N) // 4)
    # per-core block count: multiple of 3, cores cover >= NBTOT
    NB = -(-NBTOT // N_CORES)
    NB = ((NB + 2) // 3) * 3
    n_part_core = 4 * NB // 3
    # no 64-bit carry within any core's block range (checked; margin is huge)
    assert int(f0.astype(np.uint64).max()) + NB * N_CORES < 2**32, \
        "philox 64-bit counter carry in range -- unhandled"

    # step-blocks of 128
    sb = []
    s = n_steps
    while s > 0:
        sb.append(min(128, s))
        s -= 128
    nblk = len(sb)

    # free-dim chunking
    Fc_list = []
    left = NB
    while left > 0:
        Fc_list.append(min(640, left))
        left -= 640

    with np.errstate(over='ignore'):
        # host round-1/2 constant mulhilos
        lo1c, hi1c = _mulhilo_host(PHILOX_M1, f0)          # M1 on c2_0=f0
        k0 = [(f0 + np.uint32((r * PHILOX_KB0) & 0xFFFFFFFF)).astype(np.uint32) for r in range(10)]
        k1 = [(f1 + np.uint32((r * PHILOX_KB1) & 0xFFFFFFFF)).astype(np.uint32) for r in range(10)]
        c0_1 = hi1c ^ f1 ^ k0[0]
        lo0c2, hi0c2 = _mulhilo_host(PHILOX_M0, c0_1)      # M0 on const c0_1
        XC2_1 = f1 ^ k1[0]
        XC0_2 = lo1c ^ k0[1]
        XC2_2 = hi0c2 ^ k1[1]
        XC2_3 = lo0c2 ^ k1[2]

    NKC = 21
    keys_all = np.zeros((N_CORES, P, nblk * NKC), np.uint32)
    wt_all = np.zeros((N_CORES, P, nblk), np.float32)
    for c in range(N_CORES):
        for kb in range(nblk):
            lo = kb * 128
            n_in = sb[kb]
            rows = slice(0, n_in)
            stepsl = slice(lo, lo + n_in)
            col = kb * NKC
            with np.errstate(over='ignore'):
                keys_all[c, rows, col + 0] = (f0[stepsl] + np.uint32(c * NB)).astype(np.uint32)
            keys_all[c, rows, col + 1] = XC2_1[stepsl]
            keys_all[c, rows, col + 2] = XC0_2[stepsl]
            keys_all[c, rows, col + 3] = XC2_2[stepsl]
            keys_all[c, rows, col + 4] = XC2_3[stepsl]
            for r in range(2, 10):
                keys_all[c, rows, col + 5 + (r - 2)] = k0[r][stepsl]
                keys_all[c, rows, col + 13 + (r - 2)] = k1[r][stepsl]
            wt_all[c, rows, kb] = w[stepsl]

    bt_all = np.zeros((N_CORES, 65, n_meas), np.float32)
    for d in range(3):
        bt_all[:, 32 * d, :] = B.T[d][None, :]
    mask_all = np.ones((N_CORES, 65, 64), np.float32)
    pad_parts = N_CORES * n_part_core - N
    if pad_parts > 0:
        mask_all[-1, :, 64 - pad_parts:] = 0.0

    import os
    cfg = (NB, n_steps, n_part_core, tuple(Fc_list), tuple(sb), n_meas,
           os.environ.get("KDBG", "0") == "1")
    runner = _get_runner(cfg)

    feed = {"keys": keys_all, "wt": wt_all, "bt": bt_all, "mask64": mask_all}
    args = [np.concatenate([feed[k][c] for c in range(N_CORES)], axis=0)
            for k in runner["in_names"]]
    args += [np.zeros((N_CORES * z.shape[0], *z.shape[1:]), z.dtype)
             for z in runner["zero_outs"]]
    import jax
    outs = runner["fn"](*args)
    jax.block_until_ready(outs)
    oidx = runner["out_names"].index("out")
    res = np.asarray(outs[oidx]).reshape(N_CORES, n_meas, 2)
    import os as _os
    if _os.environ.get("KDBG", "0") == "1":
        dbgres = {}
        for i, nm in enumerate(runner["out_names"]):
            if nm.startswith("dbg_"):
                av = runner["out_avals"][i]
                dbgres[nm] = np.asarray(outs[i]).reshape(N_CORES, *av.shape)
        np.savez("/root/problem/dbg_dump.npz", **dbgres)

    cos_sum = res[:, :, 0].sum(axis=0) - np.float64(pad_parts)
    sin_sum = res[:, :, 1].sum(axis=0)
    c = cos_sum / N
    s = sin_sum / N
    return np.sqrt(c * c + s * s).astype(np.float32)
